# revision 26
# baseline (speedup 1.0000x reference)
"""Trainium2 Bass kernel for a custom LSTM cell step.

Reference computation (per full problem, B=8192, D=U=512):
    z = inputs @ kernel + h_tm1 @ recurrent_kernel + bias        # [B, 4U]
    i, f, g, o = split(z, 4, axis=1)
    i, f, o = sigmoid(...)  ;  g = tanh(g)
    c = f * c_tm1 + i * g
    h = o * tanh(c)
    return (h, h, c)

Sharding: data-parallel over the batch dim across 8 NeuronCores
(1024 rows per core); kernel/recurrent_kernel/bias replicated.

Per-core kernel structure (v5 — fp16 stream + fp8 DoubleRow i- and f-gates):
  - Inputs travel as fp16, pre-transposed to [k, m] on the host (no PE
    transposes); outputs return as fp16 and are upcast on the host.
  - The PE is the bottleneck: a 16-bit N=512 matmul costs 216 ns (1
    output-row/cycle @ 2.4 GHz), 256 of them = 55.4 us.  The i-gate
    and f-gates are computed in fp8-e4m3 with perf_mode=DoubleRow (2
    MACs/cell/cycle, k-pairs packed along the free dim), cutting their
    128 matmuls to 64 at ~2x rate.  Numerics: fp8 on the two sigmoid
    gates i/f lands h 1.75e-2 / c 1.83e-2 rel-err (Monte-Carlo,
    validated exactly against HW for the fp16 and fp8-i variants) vs
    the 2e-2 budget; fp8 on g or o as well would exceed it.
  - ALL fp16 inputs are packed by the host into ONE [128, 48, 512]
    tensor in consumption order (xh0/Wg interleaved by k for the ramp,
    then xh1, Wf, c, Wo); the fp8 payload (xh + Wi, k-paired) is a
    second [128, 8, 1536] tensor DMA'd mid-stream.  Every chunk is
    [:, a:b, :] = 128 x contiguous-KB descriptors.
  - Phase order g, i(fp8), f, o tracks DMA arrivals; warmup matmuls
    release the HAM clock gate while the first chunks stream in.
  - Post-PSUM elementwise runs in fp16 (2x DVE rate); the last m-tile's
    o-phase runs in two N=256 halves to shorten the ACT/DVE/store tail.
"""

from contextlib import ExitStack

import ml_dtypes
import numpy as np

import concourse.bass as bass
import concourse.mybir as mybir
import concourse.tile as tile
from concourse import bacc
from concourse.bass_utils import run_bass_kernel_spmd

# Problem sizes (hardcoded per spec).
B, D, U = 8192, 512, 512
N_CORES = 8
MB = B // N_CORES  # 1024 batch rows per core
P = 128
MT = MB // P  # 8 m-tiles per core
KO = (D + U) // P  # 8 stacked contraction tiles (4 from W/x, 4 from R/h)
NG = 4 * U  # 2048 gate columns
NBLK = 40  # fp16 stream blocks of [P, 512] (128 KB each)

F32 = mybir.dt.float32
F16 = mybir.dt.float16
F8 = mybir.dt.float8e4
NP16 = np.float16
NP8 = ml_dtypes.float8_e4m3
DR = mybir.MatmulPerfMode.DoubleRow

SIG = mybir.ActivationFunctionType.Sigmoid
TANH = mybir.ActivationFunctionType.Tanh

_NC_CACHE: dict = {}


def _xh_blk(mh, ko):
    return 2 * ko if mh == 0 else 16 + ko


def _w_blk(g, ko):
    # gate g -> interleaved with xh0, o -> blocks 32-39 (i/f are fp8).
    assert g in (2, 3)
    return 2 * ko + 1 if g == 2 else 32 + ko


def _c_blk(mt):
    return 24 + mt


def _build_lstm_nc(with_bias: bool):
    """Build and compile the per-core Bass program."""
    nc = bacc.Bacc("TRN2", target_bir_lowering=False, debug=False)

    st_d = nc.dram_tensor("stream", [P, NBLK, 512], F16, kind="ExternalInput")
    st8_d = nc.dram_tensor("stream8", [P, KO, 2048], F8, kind="ExternalInput")
    b_d = None
    if with_bias:
        b_d = nc.dram_tensor("bias", [NG], F32, kind="ExternalInput")
    ho_d = nc.dram_tensor("h_out", [MB, U], F16, kind="ExternalOutput")
    co_d = nc.dram_tensor("c_out", [MB, U], F16, kind="ExternalOutput")

    ho_v = ho_d.ap().rearrange("(mt p) d -> p mt d", p=P)
    co_v = co_d.ap().rearrange("(mt p) d -> p mt d", p=P)

    with tile.TileContext(nc) as tc, ExitStack() as ctx:
        consts = ctx.enter_context(tc.tile_pool(name="consts", bufs=1))
        keep = ctx.enter_context(tc.tile_pool(name="keep", bufs=4))
        scratch = ctx.enter_context(tc.tile_pool(name="scratch", bufs=3))
        outp = ctx.enter_context(tc.tile_pool(name="outp", bufs=3))
        zpsum = ctx.enter_context(tc.tile_pool(name="zpsum", bufs=5, space="PSUM"))
        zhpsum = ctx.enter_context(tc.tile_pool(name="zhpsum", bufs=2, space="PSUM"))
        wpsum = ctx.enter_context(tc.tile_pool(name="wpsum", bufs=1, space="PSUM"))

        st = consts.tile([P, NBLK, 512], F16)
        st8 = consts.tile([P, KO, 2048], F8)

        # Warmup: zeroed fp32 stationary; each fp32 matmul issues a
        # LOW/HIGH pair (~427 ns) keeping the PE busy until the first
        # real chunk lands, so HAM is at 8/8 when the fp16 stream starts.
        wt = consts.tile([P, P], F32)
        nc.gpsimd.memset(wt[:], 0.0)
        wps = wpsum.tile([P, P], F32, tag="warm")
        for _ in range(11):
            nc.tensor.matmul(wps[:], wt[:], wt[:], start=True, stop=True,
                             skip_group_check=True)

        # DMA chunks in consumption order: 0.5 MB through the ramp-critical
        # first 2 MB (xh0/Wg k-interleaved), then 1 MB chunks; the fp8
        # payload lands mid-stream right before the i-phases need it.
        for a, b in ((0, 4), (4, 8), (8, 12), (12, 16)):
            nc.sync.dma_start(st[:, a:b, :], st_d.ap()[:, a:b, :])
        # Mid/late chunks are halved: matmul deps are slice-level, so each
        # phase starts on half-arrival instead of waiting the full MB.
        nc.sync.dma_start(st[:, 16:20, :], st_d.ap()[:, 16:20, :])  # xh1 k0-3
        nc.sync.dma_start(st[:, 20:24, :], st_d.ap()[:, 20:24, :])  # xh1 k4-7
        nc.sync.dma_start(st8[:, 0:4, :], st8_d.ap()[:, 0:4, :])    # fp8 k0-3
        nc.sync.dma_start(st8[:, 4:8, :], st8_d.ap()[:, 4:8, :])    # fp8 k4-7
        nc.sync.dma_start(st[:, 24:28, :], st_d.ap()[:, 24:28, :])  # c mt0-3
        nc.sync.dma_start(st[:, 28:32, :], st_d.ap()[:, 28:32, :])  # c mt4-7
        nc.sync.dma_start(st[:, 32:36, :], st_d.ap()[:, 32:36, :])  # Wo k0-3
        nc.sync.dma_start(st[:, 36:40, :], st_d.ap()[:, 36:40, :])  # Wo k4-7

        bias_bc = None
        if with_bias:
            assert b_d is not None
            bias_bc = consts.tile([P, NG], F32)
            b_ap = b_d.ap()
            # DMA-replicate bias across all 128 partitions (partition step 0).
            nc.gpsimd.dma_start(
                out=bias_bc,
                in_=bass.AP(tensor=b_ap.tensor, offset=b_ap.offset, ap=[[0, P], [1, NG]]),
            )

        def z_part(zp, g, mt, k0, k1, n0=0, n1=U):
            """Partial z accumulation over k-tiles [k0,k1) for gate g/m-tile mt."""
            mh, mq = mt // 4, mt % 4
            for ko in range(k0, k1):
                nc.tensor.matmul(
                    zp[:],
                    st[:, _xh_blk(mh, ko), mq * P : (mq + 1) * P],
                    st[:, _w_blk(g, ko), n0:n1],
                    start=(ko == 0),
                    stop=(ko == KO - 1),
                )

        def z_chunk(g, mt):
            """Accumulate z[:, gate g] for m-tile mt into a PSUM bank."""
            zp = zpsum.tile([P, U], F32, tag="z")
            z_part(zp, g, mt, 0, KO)
            if bias_bc is not None:
                nc.vector.tensor_add(zp[:], zp[:], bias_bc[:, g * U : (g + 1) * U])
            return zp

        def z_chunk8(g, mt):
            """i/f-gate z via fp8 DoubleRow: 4 matmuls, each eating a k-pair.

            lhsT [p, 2, 128m] / rhs [p, 2, 512n]: plane j of dim1 is
            k-tile 2a+j (contraction = both planes x 128 partitions)."""
            off = 1024 + 512 * g  # gate i at 1024, gate f at 1536
            zp = zpsum.tile([P, U], F32, tag="z")
            for a in range(4):
                nc.tensor.matmul(
                    zp[:],
                    st8[:, 2 * a : 2 * a + 2, mt * P : (mt + 1) * P],
                    st8[:, 2 * a : 2 * a + 2, off : off + 512],
                    start=(a == 0),
                    stop=(a == 3),
                    perf_mode=DR,
                )
            if bias_bc is not None:
                nc.vector.tensor_add(zp[:], zp[:], bias_bc[:, g * U : (g + 1) * U])
            return zp

        g_t, ig_t, th_t = {}, {}, {}

        def phase_g(mt, zp=None):  # g = tanh(z2)
            gt = keep.tile([P, U], F16, tag="g", bufs=8)
            nc.scalar.activation(gt[:], (zp if zp is not None else z_chunk(2, mt))[:], TANH)
            g_t[mt] = gt

        def phase_i(mt):  # i = sigmoid(z0) via fp8; ig = i*g
            it = scratch.tile([P, U], F16, tag="gact")
            nc.scalar.activation(it[:], z_chunk8(0, mt)[:], SIG)
            ig = keep.tile([P, U], F16, tag="ig")
            nc.vector.tensor_mul(ig[:], it[:], g_t.pop(mt)[:])
            ig_t[mt] = ig

        def phase_f(mt):  # f = sigmoid(z1); c = f*c_old + ig; tanh(c)
            ft = scratch.tile([P, U], F16, tag="gact")
            nc.scalar.activation(ft[:], z_chunk8(1, mt)[:], SIG)
            c_new = outp.tile([P, U], F16, tag="cnew")
            nc.vector.tensor_mul(c_new[:], ft[:], st[:, _c_blk(mt), :])
            nc.vector.tensor_add(c_new[:], c_new[:], ig_t.pop(mt)[:])
            # c-stores ride the SWDGE ring so the late h-stores don't
            # queue behind them on the sync ring (HBM writes are slow).
            nc.gpsimd.dma_start(co_v[:, mt, :], c_new[:])
            th = keep.tile([P, U], F16, tag="th")
            nc.scalar.activation(th[:], c_new[:], TANH)
            th_t[mt] = th

        def phase_o(mt):  # o = sigmoid(z3); h = o*tanh(c)
            ot = scratch.tile([P, U], F16, tag="gact")
            nc.scalar.activation(ot[:], z_chunk(3, mt)[:], SIG)
            h_new = outp.tile([P, U], F16, tag="hnew")
            nc.vector.tensor_mul(h_new[:], ot[:], th_t.pop(mt)[:])
            nc.sync.dma_start(ho_v[:, mt, :], h_new[:])

        def phase_o_split(mt):
            # Last m-tile: run o in two N=256 halves so the ACT/DVE/store
            # tail of the first half hides under the second half's matmuls.
            th = th_t.pop(mt)
            for h0 in (0, 256):
                zp = zhpsum.tile([P, 256], F32, tag="zh")
                z_part(zp, 3, mt, 0, KO, h0, h0 + 256)
                if bias_bc is not None:
                    nc.vector.tensor_add(
                        zp[:], zp[:], bias_bc[:, 3 * U + h0 : 3 * U + h0 + 256]
                    )
                ot = scratch.tile([P, 256], F16, tag="gacth")
                nc.scalar.activation(ot[:], zp[:], SIG)
                h_new = outp.tile([P, 256], F16, tag="hnewh")
                nc.vector.tensor_mul(h_new[:], ot[:], th[:, h0 : h0 + 256])
                nc.sync.dma_start(ho_v[:, mt, h0 : h0 + 256], h_new[:])

        H0 = list(range(4))
        H1 = list(range(4, MT))

        # Emission order matched to DMA arrivals.  g(0-3) accumulates in
        # k-pair rounds tracking the 0.5 MB ramp chunks; the fp8 i-phases
        # interleave so the "g" ring (8 bufs) and "ig"/"th" rings (4) bound.
        zps = {}
        for k0 in (0, 2, 4, 6):
            for mt in H0:
                if k0 == 0:
                    zps[mt] = zpsum.tile([P, U], F32, tag="z", name=f"zopen{mt}")
                z_part(zps[mt], 2, mt, k0, k0 + 2)
        for mt in H0:
            if bias_bc is not None:
                nc.vector.tensor_add(zps[mt][:], zps[mt][:], bias_bc[:, 2 * U : 3 * U])
            phase_g(mt, zps[mt])
        for mt in H1:
            phase_g(mt)
        for mt in H0:
            phase_i(mt)
        for mt in H0:
            phase_f(mt)
        for mt in H1:
            phase_i(mt)
        for mt in H0:
            phase_o(mt)
        for mt in H1:
            phase_f(mt)
        for mt in H1[:-1]:
            phase_o(mt)
        phase_o_split(H1[-1])

    nc.compile()
    return nc


def _get_nc(with_bias: bool):
    if with_bias not in _NC_CACHE:
        _NC_CACHE[with_bias] = _build_lstm_nc(with_bias)
    return _NC_CACHE[with_bias]


def _prepare_in_maps(inputs, h_tm1, c_tm1, kernel, recurrent_kernel, bias):
    x = np.asarray(inputs, dtype=np.float32)
    h = np.asarray(h_tm1, dtype=np.float32)
    c = np.asarray(c_tm1, dtype=np.float32)
    w = np.asarray(kernel, dtype=np.float32)
    r = np.asarray(recurrent_kernel, dtype=np.float32)
    b = np.ascontiguousarray(np.asarray(bias, dtype=np.float32))
    with_bias = bool(np.any(b))

    # Weights: [k=1024, n=2048] -> [gate, p, ko, n'] fp16, replicated.
    wrf = np.vstack([w, r])                                # [1024, 2048]
    wr = wrf.astype(NP16).reshape(KO, P, 4, 512).transpose(2, 1, 0, 3)
    # i-gate weights in fp8: [p, ko, n'].
    wi8 = wrf[:, 0:512].astype(NP8).reshape(KO, P, 512).transpose(1, 0, 2)
    wf8 = wrf[:, 512:1024].astype(NP8).reshape(KO, P, 512).transpose(1, 0, 2)

    in_maps = []
    for core in range(N_CORES):
        sl = slice(core * MB, (core + 1) * MB)
        xhf = np.concatenate([x[sl], h[sl]], axis=1)              # [1024m, 1024k]
        # fp16: [m, k] -> [mh, p, ko, m'].
        xh = xhf.astype(NP16).reshape(2, 512, KO, P).transpose(0, 3, 2, 1)
        # fp8: [m, k] -> [p, ko, m] (all 1024 m).
        xh8 = xhf.astype(NP8).reshape(1024, KO, P).transpose(2, 1, 0)
        # c: [m=1024, u] -> [p, mt, u] fp16.
        cc = c[sl].astype(NP16).reshape(MT, P, U).transpose(1, 0, 2)

        # Pack the fp16 stream in consumption order.
        stream = np.empty((P, NBLK, 512), dtype=NP16)
        for ko in range(KO):
            stream[:, 2 * ko, :] = xh[0, :, ko, :]      # xh0 k
            stream[:, 2 * ko + 1, :] = wr[2, :, ko, :]  # Wg k
        stream[:, 16:24, :] = xh[1]                     # xh1
        stream[:, 24:32, :] = cc                        # c
        stream[:, 32:40, :] = wr[3]                     # Wo

        # fp8 payload: per ko, [xh8 (1024 m) | wi8 (512 n) | wf8 (512 n)].
        stream8 = np.empty((P, KO, 2048), dtype=NP8)
        stream8[:, :, 0:1024] = xh8
        stream8[:, :, 1024:1536] = wi8
        stream8[:, :, 1536:2048] = wf8

        m = {"stream": stream, "stream8": stream8}
        if with_bias:
            m["bias"] = b
        in_maps.append(m)
    return in_maps, with_bias


def kernel(inputs, h_tm1, c_tm1, kernel, recurrent_kernel, bias):
    in_maps, with_bias = _prepare_in_maps(
        inputs, h_tm1, c_tm1, kernel, recurrent_kernel, bias
    )
    nc = _get_nc(with_bias)
    res = run_bass_kernel_spmd(nc, in_maps, core_ids=list(range(N_CORES)))
    h_out = np.concatenate([r_["h_out"] for r_ in res.results], axis=0).astype(np.float32)
    c_out = np.concatenate([r_["c_out"] for r_ in res.results], axis=0).astype(np.float32)
    return (h_out, h_out, c_out)
